# revision 14
# baseline (speedup 1.0000x reference)
"""Grouped single-step GRU (B=1024, U=8, I=H=512) on 8 trn2 NeuronCores.

Strategy: expert-parallel — core u computes GRU unit u for the whole batch.
Host pre-transposes operands so the device kernel is pure GEMM + fused
epilogue with zero on-chip transposes:
  - xT, hT:  [I, B]   (contraction dim on partitions for both matmul operands)
  - weights host-reordered to [J=4, 128, K*3*128] so each output j-chunk's
    weight slab is one contiguous DMA with 6KB partition lines
  - gates land as [g, b] -> per-partition biases ride free on the ACT engine.
Matmuls run as float32r (full PE rate at N=512 moving operand).
DMA-issue cost (~0.6us per dma_start on a sequencer) dominates small-tile
designs, so all loads are a handful of big DMAs spread across the two HWDGE
engines (weights on scalar, x/h + stores on sync). A burst of junk matmuls
at t=0 warms the PE HAM clock gate while the first loads stream.
Output is produced as outT [H, B] per core; host transposes back.
"""

import sys

if "/opt/trn_rl_repo" not in sys.path:
    sys.path.insert(0, "/opt/trn_rl_repo")

from contextlib import ExitStack

import numpy as np

import concourse.tile as tile
from concourse import bacc, mybir
from concourse.bass_utils import run_bass_kernel_spmd

B, U, I, H = 1024, 8, 512, 512
G = 3 * H
NB = 512          # moving-operand width (b-half)
NBH = B // NB     # 2 b-halves
KT = I // 128     # 4 contraction chunks
JT = H // 128     # 4 output-gate partition chunks
N_WARMUP_MM = 38  # ~10us of cold matmuls to flip the HAM clock gate early

F32 = mybir.dt.float32
F32R = mybir.dt.float32r
AF = mybir.ActivationFunctionType
ALU = mybir.AluOpType

LAST_EXEC_NS = None
TRACE = False
TRACE_DIR = None

_compiled = None


def _ensure_ntff_hook():
    """Provide antenv.axon_hooks + a ctypes NTFF hook when the image lacks
    them (mirrors trn_agent_boot's degraded-silently path), and keep trace
    artifacts local instead of uploading."""
    import contextlib
    import ctypes
    import types

    from concourse import bass_utils as _bu

    _bu.upload_artifacts = lambda tmpdir: f"local://{tmpdir}"

    try:
        from antenv.axon_hooks import get_axon_ntff_profile_hook  # noqa: F401

        return
    except ImportError:
        pass

    import antenv

    mod = types.ModuleType("antenv.axon_hooks")
    _holder = [None]
    mod.set_axon_ntff_profile_hook = lambda h: _holder.__setitem__(0, h)
    mod.get_axon_ntff_profile_hook = lambda: _holder[0]
    sys.modules["antenv.axon_hooks"] = mod
    antenv.axon_hooks = mod

    lib = ctypes.CDLL("/opt/axon/libaxon_pjrt.so")
    if not hasattr(lib, "axon_start_nrt_profile"):
        return
    lib.axon_start_nrt_profile.argtypes = [
        ctypes.POINTER(ctypes.c_int64),
        ctypes.c_size_t,
    ]
    lib.axon_start_nrt_profile.restype = ctypes.c_int64
    lib.axon_stop_nrt_profile.argtypes = [ctypes.c_char_p]
    lib.axon_stop_nrt_profile.restype = ctypes.c_int64

    @contextlib.contextmanager
    def _hook(output_dir, device_ids):
        import jax

        jax.devices()
        if device_ids:
            ids = (ctypes.c_int64 * len(device_ids))(*device_ids)
            rc = lib.axon_start_nrt_profile(ids, len(device_ids))
        else:
            rc = lib.axon_start_nrt_profile(None, 0)
        if rc != 0:
            raise RuntimeError(f"axon_start_nrt_profile rc={rc}")
        try:
            yield
        finally:
            n = lib.axon_stop_nrt_profile(str(output_dir).encode())
            print(f"ntff profile: {n} file(s) written to {output_dir}")

    mod.set_axon_ntff_profile_hook(_hook)


def _build():
    nc = bacc.Bacc(
        "TRN2",
        target_bir_lowering=False,
        debug=False,
        num_devices=U,
    )
    xT = nc.dram_tensor("xT", [NBH, 128, KT * NB], F32R, kind="ExternalInput").ap()
    hT = nc.dram_tensor("hT", [NBH, 128, KT * NB], F32R, kind="ExternalInput").ap()
    # weight slabs: [j, p, k*384 + g*128 + c]
    wih2 = nc.dram_tensor("wih2", [JT, 128, KT * 384], F32R, kind="ExternalInput").ap()
    whh2 = nc.dram_tensor("whh2", [JT, 128, KT * 384], F32R, kind="ExternalInput").ap()
    biases = nc.dram_tensor("biases", [128, 16], F32, kind="ExternalInput").ap()
    outT = nc.dram_tensor("outT", [H, B], F32, kind="ExternalOutput").ap()

    with tile.TileContext(nc) as tc, ExitStack() as ctx:
        wpool = ctx.enter_context(tc.tile_pool(name="w", bufs=1))
        xpool = ctx.enter_context(tc.tile_pool(name="x", bufs=1))
        bpool = ctx.enter_context(tc.tile_pool(name="b", bufs=1))
        ppool = ctx.enter_context(tc.tile_pool(name="psum", bufs=2, space="PSUM"))
        epool = ctx.enter_context(tc.tile_pool(name="work", bufs=4))

        # PE warmup: junk matmuls (zeroed operands) keep the HAM activity
        # window busy during the initial load phase so real matmuls run at
        # 2.4GHz from the start. Results land in a never-read PSUM tile.
        jnk32 = bpool.tile([128, NB], F32, tag="jnk32")
        jnkr = bpool.tile([128, NB], F32R, tag="jnkr")
        nc.vector.memset(jnk32[:], 0.0)
        nc.vector.tensor_scalar_mul(jnkr[:], jnk32[:], 0.0)
        pjnk = ppool.tile([128, NB], F32, tag="pr")
        for _ in range(N_WARMUP_MM):
            nc.tensor.matmul(
                pjnk[:], lhsT=jnkr[:, 0:128], rhs=jnkr[:], start=True, stop=True
            )

        # Every load is split in free-dim halves across BOTH HWDGE queues
        # (scalar + sync) in global consumption order, so each tensor
        # arrives at the ~440GB/s aggregate rate instead of one queue's
        # ~220GB/s — late weight slabs were starving the PE mid-kernel.
        wih_s = {}
        whh_s = {}
        x_s = {}
        h_s = {}

        def load_split(t, dram_ap):
            half = dram_ap.shape[1] // 2
            nc.scalar.dma_start(out=t[:, :half], in_=dram_ap[:, :half])
            nc.sync.dma_start(out=t[:, half:], in_=dram_ap[:, half:])

        def load_w(j):
            for d, dram, nm in ((wih_s, wih2, "wih"), (whh_s, whh2, "whh")):
                t = wpool.tile([128, KT * 384], F32R, tag=f"{nm}_{j}")
                load_split(t, dram[j])
                d[j] = t

        def load_xh(bh):
            for d, dram, nm in ((x_s, xT, "x"), (h_s, hT, "h")):
                t = xpool.tile([128, KT * NB], F32R, tag=f"{nm}_{bh}")
                load_split(t, dram[bh])
                d[bh] = t

        bt = bpool.tile([128, 16], F32, tag="bias")
        nc.scalar.dma_start(out=bt[:], in_=biases[:])
        load_w(0)
        load_xh(0)
        load_w(1)
        load_w(2)
        load_w(3)
        load_xh(1)

        def wsl(ws, j, k, g):
            return ws[j][:, k * 384 + g * 128 : k * 384 + g * 128 + 128]

        for bh in range(NBH):
            for j in range(JT):
                pr = ppool.tile([128, NB], F32, tag="pr")
                pz = ppool.tile([128, NB], F32, tag="pz")
                pxn = ppool.tile([128, NB], F32, tag="pxn")
                phn = ppool.tile([128, NB], F32, tag="phn")

                # xn first: depends only on wih + x (earliest-arriving data).
                for k in range(KT):
                    nc.tensor.matmul(
                        pxn[:],
                        lhsT=wsl(wih_s, j, k, 2),
                        rhs=x_s[bh][:, k * NB : (k + 1) * NB],
                        start=(k == 0),
                        stop=(k == KT - 1),
                    )
                # r and z gates: accumulate x@W + h@U in one PSUM group.
                for pt, g in ((pr, 0), (pz, 1)):
                    ops = [
                        (wsl(wih_s, j, k, g), x_s[bh][:, k * NB : (k + 1) * NB])
                        for k in range(KT)
                    ]
                    ops += [
                        (wsl(whh_s, j, k, g), h_s[bh][:, k * NB : (k + 1) * NB])
                        for k in range(KT)
                    ]
                    for i, (w, r) in enumerate(ops):
                        nc.tensor.matmul(
                            pt[:],
                            lhsT=w,
                            rhs=r,
                            start=(i == 0),
                            stop=(i == len(ops) - 1),
                        )
                for k in range(KT):
                    nc.tensor.matmul(
                        phn[:],
                        lhsT=wsl(whh_s, j, k, 2),
                        rhs=h_s[bh][:, k * NB : (k + 1) * NB],
                        start=(k == 0),
                        stop=(k == KT - 1),
                    )

                r_t = epool.tile([128, NB], F32, tag="r")
                z_t = epool.tile([128, NB], F32, tag="z")
                t_t = epool.tile([128, NB], F32, tag="t")
                s_t = epool.tile([128, NB], F32, tag="s")
                n_t = epool.tile([128, NB], F32, tag="n")
                d_t = epool.tile([128, NB], F32, tag="d")
                e_t = epool.tile([128, NB], F32, tag="e")
                o_t = epool.tile([128, NB], F32, tag="o")
                h_j = h_s[bh][:, j * NB : (j + 1) * NB]

                def epilogue(c0, c1):
                    cs = slice(c0, c1)
                    nc.scalar.activation(
                        r_t[:, cs], pr[:, cs], AF.Sigmoid, bias=bt[:, j : j + 1]
                    )
                    nc.scalar.activation(
                        z_t[:, cs], pz[:, cs], AF.Sigmoid, bias=bt[:, 4 + j : 5 + j]
                    )
                    # t = (hn + b_hn) * r
                    nc.vector.scalar_tensor_tensor(
                        t_t[:, cs],
                        phn[:, cs],
                        bt[:, 12 + j : 13 + j],
                        r_t[:, cs],
                        op0=ALU.add,
                        op1=ALU.mult,
                    )
                    nc.vector.tensor_tensor(
                        s_t[:, cs], t_t[:, cs], pxn[:, cs], op=ALU.add
                    )
                    nc.scalar.activation(
                        n_t[:, cs], s_t[:, cs], AF.Tanh, bias=bt[:, 8 + j : 9 + j]
                    )
                    # out = n + z * (h - n)
                    nc.vector.tensor_tensor(
                        d_t[:, cs], h_j[:, cs], n_t[:, cs], op=ALU.subtract
                    )
                    nc.vector.tensor_tensor(
                        e_t[:, cs], z_t[:, cs], d_t[:, cs], op=ALU.mult
                    )
                    nc.vector.tensor_tensor(
                        o_t[:, cs], n_t[:, cs], e_t[:, cs], op=ALU.add
                    )
                    nc.sync.dma_start(
                        out=outT[
                            j * 128 : (j + 1) * 128, bh * NB + c0 : bh * NB + c1
                        ],
                        in_=o_t[:, cs],
                    )

                if bh == NBH - 1 and j == JT - 1:
                    # last chunk: halve the serial epilogue latency
                    epilogue(0, NB // 2)
                    epilogue(NB // 2, NB)
                else:
                    epilogue(0, NB)

    nc.compile()
    return nc


def _get_nc():
    global _compiled
    if _compiled is None:
        _compiled = _build()
    return _compiled


def _prep_in_maps(inputs, hidden, W_ih, W_hh, b_ih, b_hh):
    def pack_xh(a):
        # [B, U, I] -> [U, bh, p, k*NB + b]: tile[p, k*NB+b] = a[bh*NB+b, u, k*128+p]
        a = np.asarray(a, dtype=np.float32)
        a5 = a.reshape(NBH, NB, U, KT, 128)  # [bh, b, u, k, p]
        return np.ascontiguousarray(a5.transpose(2, 0, 4, 3, 1)).reshape(
            U, NBH, 128, KT * NB
        )

    x = pack_xh(inputs)
    h = pack_xh(hidden)

    def reorder_w(W):
        # [U, G, I] -> per-unit [J, 128, K*384]: slab[j, p, k*384+g*128+c]
        # = W.T[k*128+p, g*512+j*128+c]
        wT = np.asarray(W, dtype=np.float32).transpose(0, 2, 1)  # [U, I, G]
        w5 = wT.reshape(U, KT, 128, 3, JT, 128)  # [u, k, p, g, j, c]
        return np.ascontiguousarray(w5.transpose(0, 4, 2, 1, 3, 5)).reshape(
            U, JT, 128, KT * 384
        )

    wih = reorder_w(W_ih)
    whh = reorder_w(W_hh)
    bi = np.asarray(b_ih, dtype=np.float32)
    bh = np.asarray(b_hh, dtype=np.float32)
    brz = bi[:, : 2 * H] + bh[:, : 2 * H]  # r and z biases combine
    b_in = bi[:, 2 * H :]
    b_hn = bh[:, 2 * H :]
    in_maps = []
    for u in range(U):
        # [128, 16] tile: column cls*4 + j holds bias_cls[j*128 + p]
        bb = np.stack([brz[u, :H], brz[u, H:], b_in[u], b_hn[u]], axis=0)
        bb = bb.reshape(4, 4, 128).transpose(2, 0, 1).reshape(128, 16)
        in_maps.append(
            {
                "xT": x[u],
                "hT": h[u],
                "wih2": wih[u],
                "whh2": whh[u],
                "biases": np.ascontiguousarray(bb),
            }
        )
    return in_maps


def kernel(inputs, hidden, W_ih, W_hh, b_ih, b_hh):
    global LAST_EXEC_NS
    nc = _get_nc()
    in_maps = _prep_in_maps(inputs, hidden, W_ih, W_hh, b_ih, b_hh)
    kwargs = {}
    if TRACE:
        _ensure_ntff_hook()
        if TRACE_DIR is not None:
            import os

            os.makedirs(TRACE_DIR, exist_ok=True)
            kwargs["tmpdir"] = TRACE_DIR
    res = run_bass_kernel_spmd(nc, in_maps, list(range(U)), trace=TRACE, **kwargs)
    LAST_EXEC_NS = res.exec_time_ns
    out = np.empty((B, U, H), dtype=np.float32)
    for u in range(U):
        out[:, u, :] = res.results[u]["outT"].T
    return out
